# revision 12
# baseline (speedup 1.0000x reference)
"""Trainium2 Bass kernel for nn_BoardEncoder (HexConv board encoder).

Math:
  h[b,n,:] = relu(x[b,n] @ Wc.T + sum_k neighbors[b,n,k] @ Wd[k].T + bc + bd.sum(0))
  out[b]   = h[b].reshape(216) @ Wf.T + bf

Strategy (pure data-parallel over batch, 8 cores x 2048 rows):
  - Host packs per-(b,n) token features [x | neighbors | 1.0] into a
    feature-major layout xt[n, p, c*2048 + b] = feat[c*113 + p] so the
    device streams big contiguous DMAs and the PE contracts over the
    feature dim (features on partitions), K split into 4 chunks of 113.
  - Stage 1 (per board cell n): psum[4, 512b] += Wchunk.T @ xtchunk,
    relu -> aligned SBUF strip [4, 2048], then SBUF->SBUF DMA scatters
    the strip to partition 4n of the h^T [(n,h), b] accumulator.
  - Stage 2: out[128b, 256] = hA.T @ WfT[:128] + hB.T @ WfT[128:] with a
    constant ones-row in hB providing the bf bias.
"""

import sys

sys.path.insert(0, "/opt/trn_rl_repo")

import numpy as np

B = 16384
N = 54
D_IN = 64
KN = 6
D_HID = 4
D_OUT = 256
NCORES = 8
BS = B // NCORES          # 2048 batch rows per core
F = D_IN + KN * D_IN + 1  # 449 features incl. constant-1 bias feature
CH = 113                  # K-chunk partition size (4 * 113 = 452 >= 449)
NCH = 4
FPAD = CH * NCH           # 452
BT = 512                  # stage-1 moving free dim (tokens per matmul)
NBT = BS // BT            # 4

LAST_EXEC_NS = None

_PROGRAM = None


def _build_program(reps=1, mode="plain"):
    """mode: "plain" (exact fp32), "f32r" (fast fp32 PE streaming), or
    "coltile2" (exact fp32, pairs of cells run concurrently in separate
    32-column PE groups). reps>1 unrolls the computation for benchmarking."""
    import concourse.bacc as bacc
    import concourse.tile as tile
    from concourse import mybir

    f32 = mybir.dt.float32
    s1dt = mybir.dt.float32r if mode == "f32r" else f32

    nc = bacc.Bacc("TRN2", target_bir_lowering=False, debug=False,
                   num_devices=NCORES)
    xt_d = nc.declare_dram_parameter("xt", [N, CH, NCH * BS], s1dt,
                                     isOutput=False)
    w_d = nc.declare_dram_parameter("w", [CH, NCH * D_HID], s1dt,
                                    isOutput=False)
    wfta_d = nc.declare_dram_parameter("wfta", [128, D_OUT], f32,
                                       isOutput=False)
    wftb_d = nc.declare_dram_parameter("wftb", [89, D_OUT], f32,
                                       isOutput=False)
    out_d = nc.declare_dram_parameter("out", [BS, D_OUT], f32, isOutput=True)

    with tile.TileContext(nc) as tc:
        with (
            tc.tile_pool(name="consts", bufs=1) as consts,
            tc.tile_pool(name="hacc", bufs=1) as hacc,
            tc.tile_pool(name="xt", bufs=4) as xtp,
            tc.tile_pool(name="hn", bufs=4 if mode == "coltile2" else 3) as hnp,
            tc.tile_pool(name="ps1", bufs=4, space="PSUM") as ps1,
            tc.tile_pool(name="ps2", bufs=2, space="PSUM") as ps2,
            tc.tile_pool(name="outp", bufs=3) as outp,
        ):
            w_sb = consts.tile([CH, NCH * D_HID], s1dt, tag="w")
            nc.sync.dma_start(w_sb[:], w_d[:])
            wfta_sb = consts.tile([128, D_OUT], f32, tag="wfta")
            nc.sync.dma_start(wfta_sb[:], wfta_d[:])
            wftb_sb = consts.tile([89, D_OUT], f32, tag="wftb")
            nc.sync.dma_start(wftb_sb[:], wftb_d[:])

            for rep in range(reps):
                hA = hacc.tile([128, BS], f32, tag="hA")  # (n,h) rows 0..127
                hB = hacc.tile([89, BS], f32, tag="hB")   # rows 128..215+ones
                # rows 0..87 are overwritten by the per-cell scatter DMAs
                # below; row 88 keeps the 1.0 fill and provides the bf bias
                # in stage 2. (a [88:89] memset is rejected: compute-engine
                # partition bases must be 32-aligned)
                nc.vector.memset(hB[:, :], 1.0)

                def load_xt(xt, n):
                    # One dma_start is serviced by a single SDMA engine
                    # (~27.5 GB/s); aggregate BW scales with the number of
                    # outstanding DMAs. Issue 4 partition-sliced sub-DMAs
                    # per tile (4 bufs x 4 slices = 16 outstanding -> HBM
                    # bound). Slices stay ~1 MB / ~29 descriptors so Q7
                    # descriptor emission (~5 us per 113-desc dma_start)
                    # doesn't become the cap.
                    for p0, p1 in ((0, 29), (29, 58), (58, 87), (87, CH)):
                        nc.gpsimd.dma_start(xt[p0:p1, :], xt_d[n, p0:p1, :])

                def scatter(n, hn):
                    # scatter on sync/HWDGE: its sem-wait on the relu must
                    # not block the big-input-load FIFO (loads live on the
                    # gpsimd/SWDGE path, which also spreads each copy across
                    # all 16 SDMA engines; HWDGE dynamic is ~1 engine)
                    if n < 32:
                        nc.sync.dma_start(hA[n * 4:(n + 1) * 4, :], hn[:])
                    else:
                        m = n - 32
                        nc.sync.dma_start(hB[m * 4:(m + 1) * 4, :], hn[:])

                if mode == "coltile2":
                    for g in range(N // 2):
                        xts = []
                        hns = []
                        for j in range(2):
                            xt = xtp.tile([CH, NCH * BS], s1dt)
                            load_xt(xt, 2 * g + j)
                            xts.append(xt)
                            hns.append(hnp.tile([D_HID, BS], f32))
                        for bt in range(NBT):
                            ps = ps1.tile([64, BT], f32)
                            for c in range(NCH):
                                for j in range(2):
                                    nc.tensor.matmul(
                                        ps[32 * j:32 * j + D_HID, :],
                                        w_sb[:, c * D_HID:(c + 1) * D_HID],
                                        xts[j][:, c * BS + bt * BT:
                                               c * BS + (bt + 1) * BT],
                                        start=(c == 0),
                                        stop=(c == NCH - 1),
                                        tile_position=(0, 32 * j),
                                    )
                            for j in range(2):
                                dst = hns[j][:, bt * BT:(bt + 1) * BT]
                                src = ps[32 * j:32 * j + D_HID, :]
                                if j == 0:
                                    nc.vector.tensor_scalar_max(dst, src, 0.0)
                                else:
                                    nc.scalar.activation(
                                        dst, src,
                                        mybir.ActivationFunctionType.Relu)
                        for j in range(2):
                            scatter(2 * g + j, hns[j])
                else:
                    for n in range(N):
                        xt = xtp.tile([CH, NCH * BS], s1dt)
                        load_xt(xt, n)
                        hn = hnp.tile([D_HID, BS], f32)
                        for bt in range(NBT):
                            ps = ps1.tile([D_HID, BT], f32)
                            for c in range(NCH):
                                nc.tensor.matmul(
                                    ps[:],
                                    w_sb[:, c * D_HID:(c + 1) * D_HID],
                                    xt[:, c * BS + bt * BT:
                                       c * BS + (bt + 1) * BT],
                                    start=(c == 0),
                                    stop=(c == NCH - 1),
                                )
                            dst = hn[:, bt * BT:(bt + 1) * BT]
                            if n % 2 == 0:
                                nc.vector.tensor_scalar_max(dst, ps[:], 0.0)
                            else:
                                nc.scalar.activation(
                                    dst, ps[:],
                                    mybir.ActivationFunctionType.Relu)
                        scatter(n, hn)

                for t in range(BS // 128):
                    po = ps2.tile([128, D_OUT], f32)
                    nc.tensor.matmul(po[:], hA[:, t * 128:(t + 1) * 128],
                                     wfta_sb[:], start=True, stop=False)
                    nc.tensor.matmul(po[:], hB[:, t * 128:(t + 1) * 128],
                                     wftb_sb[:], start=False, stop=True)
                    ot = outp.tile([128, D_OUT], f32)
                    if t % 2 == 0:
                        nc.vector.tensor_copy(ot[:], po[:])
                    else:
                        nc.scalar.copy(ot[:], po[:])
                    nc.sync.dma_start(out_d[t * 128:(t + 1) * 128, :], ot[:])

    nc.compile()
    return nc


def _get_program():
    global _PROGRAM
    if _PROGRAM is None:
        _PROGRAM = _build_program()
    return _PROGRAM


def _pack_inputs(x, neighbors):
    """Per-shard feature-major packing: xt[n, p, c*BS + b] = feat[c*113+p]
    of batch row (shard*BS + b), cell n. feat = [x | neighbors | 1 | 0pad]."""
    xts = []
    tmp = np.empty((BS, FPAD), np.float32)
    tmp[:, F - 1] = 1.0
    tmp[:, F:] = 0.0
    for s in range(NCORES):
        sl = slice(s * BS, (s + 1) * BS)
        xt = np.empty((N, CH, NCH * BS), np.float32)
        xv = xt.reshape(N, CH, NCH, BS)
        xs = x[sl]
        ns = neighbors[sl].reshape(BS, N, KN * D_IN)
        for n in range(N):
            tmp[:, :D_IN] = xs[:, n, :]
            tmp[:, D_IN:F - 1] = ns[:, n, :]
            for c in range(NCH):
                xv[n, :, c, :] = tmp[:, c * CH:(c + 1) * CH].T
        xts.append(xt)
    return xts


def _pack_weights(Wc, bc, Wd, bd, Wf, bf):
    W_all = np.zeros((FPAD, D_HID), np.float32)
    W_all[:D_IN] = Wc.T
    W_all[D_IN:F - 1] = Wd.transpose(0, 2, 1).reshape(KN * D_IN, D_HID)
    W_all[F - 1] = bc + bd.sum(0)
    # w[p, c*4+h] = W_all[c*113+p, h]
    w = np.ascontiguousarray(
        W_all.reshape(NCH, CH, D_HID).transpose(1, 0, 2)).reshape(
            CH, NCH * D_HID)
    WfT = np.ascontiguousarray(Wf.T)            # [216, 256]
    wfta = np.ascontiguousarray(WfT[:128])
    wftb = np.concatenate([WfT[128:], bf[None, :]], axis=0)  # [89, 256]
    wftb = np.ascontiguousarray(wftb.astype(np.float32))
    return w, wfta, wftb


def kernel(x, neighbors, Wc, bc, Wd, bd, Wf, bf):
    global LAST_EXEC_NS
    from concourse.bass_utils import run_bass_kernel_spmd

    x = np.asarray(x, np.float32)
    neighbors = np.asarray(neighbors, np.float32)
    w, wfta, wftb = _pack_weights(
        np.asarray(Wc, np.float32), np.asarray(bc, np.float32),
        np.asarray(Wd, np.float32), np.asarray(bd, np.float32),
        np.asarray(Wf, np.float32), np.asarray(bf, np.float32))
    xts = _pack_inputs(x, neighbors)

    nc = _get_program()
    in_maps = [
        {"xt": xts[s], "w": w, "wfta": wfta, "wftb": wftb}
        for s in range(NCORES)
    ]
    res = run_bass_kernel_spmd(nc, in_maps, list(range(NCORES)))
    LAST_EXEC_NS = res.exec_time_ns
    out = np.concatenate([res.results[s]["out"] for s in range(NCORES)],
                         axis=0)
    return out


# revision 13
# speedup vs baseline: 53.4268x; 53.4268x over previous
"""Trainium2 Bass kernel for nn_BoardEncoder (HexConv board encoder).

Math:
  h[b,n,:] = relu(x[b,n] @ Wc.T + sum_k neighbors[b,n,k] @ Wd[k].T + bc + bd.sum(0))
  out[b]   = h[b].reshape(216) @ Wf.T + bf

Strategy (pure data-parallel over batch, 8 cores x 2048 rows):
  - Host packs per-(b,n) token features [x | neighbors | 1.0] into a
    feature-major layout xt[n, p, c*2048 + b] = feat[c*113 + p] so the
    device streams big contiguous DMAs and the PE contracts over the
    feature dim (features on partitions), K split into 4 chunks of 113.
  - Stage 1 (per board cell n): psum[4, 512b] += Wchunk.T @ xtchunk,
    relu -> aligned SBUF strip [4, 2048], then SBUF->SBUF DMA scatters
    the strip to partition 4n of the h^T [(n,h), b] accumulator.
  - Stage 2: out[128b, 256] = hA.T @ WfT[:128] + hB.T @ WfT[128:] with a
    constant ones-row in hB providing the bf bias.
"""

import sys

sys.path.insert(0, "/opt/trn_rl_repo")

import numpy as np

B = 16384
N = 54
D_IN = 64
KN = 6
D_HID = 4
D_OUT = 256
NCORES = 8
BS = B // NCORES          # 2048 batch rows per core
F = D_IN + KN * D_IN + 1  # 449 features incl. constant-1 bias feature
CH = 113                  # K-chunk partition size (4 * 113 = 452 >= 449)
NCH = 4
FPAD = CH * NCH           # 452
BT = 512                  # stage-1 moving free dim (tokens per matmul)
NBT = BS // BT            # 4

LAST_EXEC_NS = None

_PROGRAM = None


def _build_program(reps=1, mode="plain"):
    """mode: "plain" (exact fp32), "f32r" (fast fp32 PE streaming), or
    "coltile2" (exact fp32, pairs of cells run concurrently in separate
    32-column PE groups). reps>1 unrolls the computation for benchmarking."""
    import concourse.bacc as bacc
    import concourse.tile as tile
    from concourse import mybir

    f32 = mybir.dt.float32
    s1dt = mybir.dt.float32r if mode == "f32r" else f32

    nc = bacc.Bacc("TRN2", target_bir_lowering=False, debug=False,
                   num_devices=NCORES)
    xt_d = nc.declare_dram_parameter("xt", [N, CH, NCH * BS], s1dt,
                                     isOutput=False)
    w_d = nc.declare_dram_parameter("w", [CH, NCH * D_HID], s1dt,
                                    isOutput=False)
    wfta_d = nc.declare_dram_parameter("wfta", [128, D_OUT], f32,
                                       isOutput=False)
    wftb_d = nc.declare_dram_parameter("wftb", [89, D_OUT], f32,
                                       isOutput=False)
    out_d = nc.declare_dram_parameter("out", [BS, D_OUT], f32, isOutput=True)

    with tile.TileContext(nc) as tc:
        with (
            tc.tile_pool(name="consts", bufs=1) as consts,
            tc.tile_pool(name="hacc", bufs=1) as hacc,
            tc.tile_pool(name="xt", bufs=4) as xtp,
            tc.tile_pool(name="hn", bufs=4 if mode == "coltile2" else 3) as hnp,
            tc.tile_pool(name="ps1", bufs=4, space="PSUM") as ps1,
            tc.tile_pool(name="ps2", bufs=2, space="PSUM") as ps2,
            tc.tile_pool(name="outp", bufs=3) as outp,
        ):
            w_sb = consts.tile([CH, NCH * D_HID], s1dt, tag="w")
            nc.sync.dma_start(w_sb[:], w_d[:])
            wfta_sb = consts.tile([128, D_OUT], f32, tag="wfta")
            nc.sync.dma_start(wfta_sb[:], wfta_d[:])
            wftb_sb = consts.tile([89, D_OUT], f32, tag="wftb")
            nc.sync.dma_start(wftb_sb[:], wftb_d[:])

            for rep in range(reps):
                hA = hacc.tile([128, BS], f32, tag="hA")  # (n,h) rows 0..127
                hB = hacc.tile([89, BS], f32, tag="hB")   # rows 128..215+ones
                # rows 0..87 are overwritten by the per-cell scatter DMAs
                # below; row 88 keeps the 1.0 fill and provides the bf bias
                # in stage 2. (a [88:89] memset is rejected: compute-engine
                # partition bases must be 32-aligned)
                nc.vector.memset(hB[:, :], 1.0)

                def load_xt(xt, n):
                    # One dma_start is serviced by a single SDMA engine
                    # (~27.5 GB/s); aggregate BW scales with the number of
                    # outstanding DMAs. Issue 4 partition-sliced sub-DMAs
                    # per tile (4 bufs x 4 slices = 16 outstanding -> HBM
                    # bound). Slices stay ~1 MB / ~29 descriptors so Q7
                    # descriptor emission (~5 us per 113-desc dma_start)
                    # doesn't become the cap.
                    for p0, p1 in ((0, 29), (29, 58), (58, 87), (87, CH)):
                        nc.gpsimd.dma_start(xt[p0:p1, :], xt_d[n, p0:p1, :])

                def scatter(n, hn):
                    # scatter on sync/HWDGE: its sem-wait on the relu must
                    # not block the big-input-load FIFO (loads live on the
                    # gpsimd/SWDGE path, which also spreads each copy across
                    # all 16 SDMA engines; HWDGE dynamic is ~1 engine)
                    if n < 32:
                        nc.sync.dma_start(hA[n * 4:(n + 1) * 4, :], hn[:])
                    else:
                        m = n - 32
                        nc.sync.dma_start(hB[m * 4:(m + 1) * 4, :], hn[:])

                if mode == "coltile2":
                    for g in range(N // 2):
                        xts = []
                        hns = []
                        for j in range(2):
                            xt = xtp.tile([CH, NCH * BS], s1dt, tag="xt")
                            load_xt(xt, 2 * g + j)
                            xts.append(xt)
                            hns.append(hnp.tile([D_HID, BS], f32, tag="hn"))
                        for bt in range(NBT):
                            ps = ps1.tile([64, BT], f32)
                            for c in range(NCH):
                                for j in range(2):
                                    nc.tensor.matmul(
                                        ps[32 * j:32 * j + D_HID, :],
                                        w_sb[:, c * D_HID:(c + 1) * D_HID],
                                        xts[j][:, c * BS + bt * BT:
                                               c * BS + (bt + 1) * BT],
                                        start=(c == 0),
                                        stop=(c == NCH - 1),
                                        tile_position=(0, 32 * j),
                                    )
                            for j in range(2):
                                dst = hns[j][:, bt * BT:(bt + 1) * BT]
                                src = ps[32 * j:32 * j + D_HID, :]
                                if j == 0:
                                    nc.vector.tensor_scalar_max(dst, src, 0.0)
                                else:
                                    nc.scalar.activation(
                                        dst, src,
                                        mybir.ActivationFunctionType.Relu)
                        for j in range(2):
                            scatter(2 * g + j, hns[j])
                else:
                    for n in range(N):
                        xt = xtp.tile([CH, NCH * BS], s1dt)
                        load_xt(xt, n)
                        hn = hnp.tile([D_HID, BS], f32)
                        for bt in range(NBT):
                            ps = ps1.tile([D_HID, BT], f32)
                            for c in range(NCH):
                                nc.tensor.matmul(
                                    ps[:],
                                    w_sb[:, c * D_HID:(c + 1) * D_HID],
                                    xt[:, c * BS + bt * BT:
                                       c * BS + (bt + 1) * BT],
                                    start=(c == 0),
                                    stop=(c == NCH - 1),
                                )
                            dst = hn[:, bt * BT:(bt + 1) * BT]
                            if n % 2 == 0:
                                nc.vector.tensor_scalar_max(dst, ps[:], 0.0)
                            else:
                                nc.scalar.activation(
                                    dst, ps[:],
                                    mybir.ActivationFunctionType.Relu)
                        scatter(n, hn)

                for t in range(BS // 128):
                    po = ps2.tile([128, D_OUT], f32)
                    nc.tensor.matmul(po[:], hA[:, t * 128:(t + 1) * 128],
                                     wfta_sb[:], start=True, stop=False)
                    nc.tensor.matmul(po[:], hB[:, t * 128:(t + 1) * 128],
                                     wftb_sb[:], start=False, stop=True)
                    ot = outp.tile([128, D_OUT], f32)
                    if t % 2 == 0:
                        nc.vector.tensor_copy(ot[:], po[:])
                    else:
                        nc.scalar.copy(ot[:], po[:])
                    nc.sync.dma_start(out_d[t * 128:(t + 1) * 128, :], ot[:])

    nc.compile()
    return nc


def _get_program():
    global _PROGRAM
    if _PROGRAM is None:
        _PROGRAM = _build_program()
    return _PROGRAM


def _pack_inputs(x, neighbors):
    """Per-shard feature-major packing: xt[n, p, c*BS + b] = feat[c*113+p]
    of batch row (shard*BS + b), cell n. feat = [x | neighbors | 1 | 0pad]."""
    xts = []
    tmp = np.empty((BS, FPAD), np.float32)
    tmp[:, F - 1] = 1.0
    tmp[:, F:] = 0.0
    for s in range(NCORES):
        sl = slice(s * BS, (s + 1) * BS)
        xt = np.empty((N, CH, NCH * BS), np.float32)
        xv = xt.reshape(N, CH, NCH, BS)
        xs = x[sl]
        ns = neighbors[sl].reshape(BS, N, KN * D_IN)
        for n in range(N):
            tmp[:, :D_IN] = xs[:, n, :]
            tmp[:, D_IN:F - 1] = ns[:, n, :]
            for c in range(NCH):
                xv[n, :, c, :] = tmp[:, c * CH:(c + 1) * CH].T
        xts.append(xt)
    return xts


def _pack_weights(Wc, bc, Wd, bd, Wf, bf):
    W_all = np.zeros((FPAD, D_HID), np.float32)
    W_all[:D_IN] = Wc.T
    W_all[D_IN:F - 1] = Wd.transpose(0, 2, 1).reshape(KN * D_IN, D_HID)
    W_all[F - 1] = bc + bd.sum(0)
    # w[p, c*4+h] = W_all[c*113+p, h]
    w = np.ascontiguousarray(
        W_all.reshape(NCH, CH, D_HID).transpose(1, 0, 2)).reshape(
            CH, NCH * D_HID)
    WfT = np.ascontiguousarray(Wf.T)            # [216, 256]
    wfta = np.ascontiguousarray(WfT[:128])
    wftb = np.concatenate([WfT[128:], bf[None, :]], axis=0)  # [89, 256]
    wftb = np.ascontiguousarray(wftb.astype(np.float32))
    return w, wfta, wftb


def kernel(x, neighbors, Wc, bc, Wd, bd, Wf, bf):
    global LAST_EXEC_NS
    from concourse.bass_utils import run_bass_kernel_spmd

    x = np.asarray(x, np.float32)
    neighbors = np.asarray(neighbors, np.float32)
    w, wfta, wftb = _pack_weights(
        np.asarray(Wc, np.float32), np.asarray(bc, np.float32),
        np.asarray(Wd, np.float32), np.asarray(bd, np.float32),
        np.asarray(Wf, np.float32), np.asarray(bf, np.float32))
    xts = _pack_inputs(x, neighbors)

    nc = _get_program()
    in_maps = [
        {"xt": xts[s], "w": w, "wfta": wfta, "wftb": wftb}
        for s in range(NCORES)
    ]
    res = run_bass_kernel_spmd(nc, in_maps, list(range(NCORES)))
    LAST_EXEC_NS = res.exec_time_ns
    out = np.concatenate([res.results[s]["out"] for s in range(NCORES)],
                         axis=0)
    return out
